# revision 27
# baseline (speedup 1.0000x reference)
"""CrossLayerTranscoder with global batch-wise top-k masking on 8 TRN2 cores.

Reference computation:
    pre = relu(x @ W_enc + b_enc)            [4096, 16384]
    keep the global top-(top_k * 4096) entries, zero the rest.

Device algorithm (dict-sharded over 8 cores), per [128 cols x 512 rows] tile:
  * PE: GEMM in fp8(e4m3) with perf_mode=DoubleRow - each matmul absorbs
    K=256 contraction rows at ~2 rows/cycle, halving PE time vs bf16.
    W is pre-scaled by 32 (power of two) to center its values in the fp8
    normal range; the ACT scale divides it back out.  Differential noise
    on z is ~0.05 rms - all accuracy-critical values are recomputed
    exactly on the host.
  * ACT: a = 8*psum + (M + 256*b) stored f32; with M = 1.5*2^23 the store
    rounds to M + q exactly (ulp(M)=1), q = round(256*(z+b)).
  * pk = (a - M) + i/512: value-major, index-minor packing, exact in f32.
    Engine-split by d-parity to balance load: even d-tiles run it as one
    DVE scalar_tensor_tensor; odd d-tiles run ACT (a - M) then GPSIMD
    tensor_tensor (+i/512), freeing DVE for the MAX8 scans.  The three
    postprocess engines (ACT/DVE/GPSIMD) measure ~115/116/104us busy -
    the steady state is their balance point and paces the MM stream.
  * DVE: MAX8 per [128 cols x 512 rows] tile extracts the top-8 packed
    values per (dict col, 512-row block).
  * w8 is d-tile-major on the host so every weight DMA slice and every
    DoubleRow LDWEIGHTS read is contiguous per partition (a strided head
    transfer otherwise delays the first matmul by ~5us).
  * Host merge:
      - decode q+b_q = floor(packed), i = frac*512; v_hat = floor/256.
      - tau_hat = k-th largest candidate.
      - candidate pool: all candidates with v_hat >= tau_hat - DELTA, plus
        every entry of 'saturated' chunks (8th extracted value still >=
        tau_hat - DELTA).  Saturated chunks are recomputed with
        per-row-block BLAS GEMMs; the rest of the pool with a chunked
        einsum.  All pool values are EXACT (f64): both the selected set
        and the stored values come from exact arithmetic, so fp8 device
        noise never reaches the output.
"""

import numpy as np

P = 128
N_TOTAL = 4096
K_DIM = 768
DICT = 16384
N_CORES = 8
DICT_SH = DICT // N_CORES     # 2048
R_BLK = 512
R_BLOCKS = N_TOTAL // R_BLK   # 8
D_TILES = DICT_SH // P        # 16
D_PAIRS = D_TILES // 2        # 8
CW = 8                        # top-8 per (col, 512-row block)
KP = K_DIM // 256             # 3 DoubleRow k-pairs
DELTA = 0.26                  # band half-width (~5 sigma of fp8 noise)
MROUND = 1.5 * 2.0**23        # fp32 round-to-int magic constant
QSCALE = 256.0                # value quantization: q = round(256*z)
W_SCALE = 32.0                # fp8 pre-scale on W (power of 2)
GPS_MOD = 2                   # odd d-tiles pack on ACT+GPSIMD, even on DVE

_cache = {}


def _build_sparse():
    import concourse.mybir as mybir
    import concourse.tile as tile
    from concourse import bacc

    f32 = mybir.dt.float32
    fp8 = mybir.dt.float8e4

    nc = bacc.Bacc("TRN2", target_bir_lowering=False, debug=False,
                   num_devices=N_CORES)
    # host layouts (partition-major):
    #   x8[p, rb, c2, ko, rr]  = fp8(x[c2*256 + ko*128 + p, rb*512 + rr])
    #   w8[p, d, c2, ko, m]    = fp8(32 * W[c2*256 + ko*128 + p, d*128 + m])
    x8 = nc.dram_tensor("x8", [P, R_BLOCKS * KP * 2 * R_BLK], fp8,
                        kind="ExternalInput")
    w8 = nc.dram_tensor("w8", [P, KP * 2 * DICT_SH], fp8,
                        kind="ExternalInput")
    # aux[:, 0:2] = [+M, -M]; [:, 2:18] = per-d ACT bias; [:, 18:530] = iota
    aux = nc.dram_tensor("aux", [P, 2 + D_TILES + R_BLK], f32,
                         kind="ExternalInput")
    cval = nc.dram_tensor("cval", [R_BLOCKS * P, D_TILES * CW], f32,
                          kind="ExternalOutput")

    with tile.TileContext(nc) as tc:
        with (
            tc.tile_pool(name="resident", bufs=1) as rpool,
            tc.tile_pool(name="xstream", bufs=4) as xpool,
            tc.tile_pool(name="act", bufs=10) as apool,
            tc.tile_pool(name="pack", bufs=10) as ppool,
            tc.tile_pool(name="cand", bufs=2) as cpool,
            tc.tile_pool(name="psum", bufs=8, space="PSUM") as psum_pool,
        ):
            # w8 is d-tile-major: [p][d][c2][ko][m] - every DMA slice and
            # every LDWEIGHTS read is contiguous per partition
            w8_sb = rpool.tile([P, D_TILES, KP, 2, P], fp8)
            aux_sb = rpool.tile([P, 2 + D_TILES + R_BLK], f32)
            mm_sb = aux_sb[:, 0:2]
            b_sb = aux_sb[:, 2:2 + D_TILES]
            io_sb = aux_sb[:, 2 + D_TILES:]

            x8_r = x8.ap().rearrange("p (rb c k rr) -> p rb c k rr",
                                     c=KP, k=2, rr=R_BLK)
            w8_r = w8.ap().rearrange("p (d c k m) -> p d c k m",
                                     d=D_TILES, c=KP, k=2)
            cval_r = cval.ap().rearrange("(rb p) w -> p rb w", p=P)

            # priority-ordered head: first MMs need w8[d=0,1] and x8(r0)
            x0 = xpool.tile([P, KP, 2, R_BLK], fp8, tag="xh")
            nc.sync.dma_start(x0[:, 0], x8_r[:, 0, 0])
            nc.sync.dma_start(w8_sb[:, 0:2], w8_r[:, 0:2])
            nc.sync.dma_start(x0[:, 1:], x8_r[:, 0, 1:])
            nc.sync.dma_start(aux_sb[:], aux.ap())
            nc.sync.dma_start(w8_sb[:, 2:4], w8_r[:, 2:4])
            nc.sync.dma_start(w8_sb[:, 4:8], w8_r[:, 4:8])
            nc.sync.dma_start(w8_sb[:, 8:], w8_r[:, 8:])

            x_next = {0: x0}
            for r in range(R_BLOCKS):
                xh_t = x_next.pop(r)
                if r + 1 < R_BLOCKS:
                    xn = xpool.tile([P, KP, 2, R_BLK], fp8, tag="xh")
                    nc.sync.dma_start(xn[:], x8_r[:, r + 1])
                    x_next[r + 1] = xn
                cvb = cpool.tile([P, D_TILES, CW], f32, tag="cv")
                for d in range(D_TILES):
                    ps = psum_pool.tile([P, R_BLK], mybir.dt.float32)
                    for c2 in range(KP):
                        nc.tensor.matmul(
                            ps[:], w8_sb[:, d, c2], xh_t[:, c2],
                            start=(c2 == 0), stop=(c2 == KP - 1),
                            perf_mode=mybir.MatmulPerfMode.DoubleRow)
                    a_sb = apool.tile([P, R_BLK], f32, tag="a")
                    nc.scalar.activation(
                        a_sb[:], ps[:],
                        mybir.ActivationFunctionType.Identity,
                        bias=b_sb[:, d:d + 1], scale=QSCALE / W_SCALE)
                    pk = ppool.tile([P, R_BLK], f32, tag="pk")
                    iosl = io_sb[:]
                    if d % GPS_MOD == 1:
                        # M removed on ACT (or DVE tensor_scalar for d=15,
                        # easing ACT), GPSIMD adds the iota
                        a2 = apool.tile([P, R_BLK], f32, tag="a2")
                        if d == 15:
                            nc.vector.tensor_scalar(
                                a2[:], a_sb[:], -MROUND, None,
                                op0=mybir.AluOpType.add)
                        else:
                            nc.scalar.activation(
                                a2[:], a_sb[:],
                                mybir.ActivationFunctionType.Identity,
                                bias=mm_sb[:, 1:2], scale=1.0)
                        nc.gpsimd.tensor_tensor(
                            pk[:], a2[:], iosl, op=mybir.AluOpType.add)
                    else:
                        nc.vector.scalar_tensor_tensor(
                            pk[:], a_sb[:], MROUND, iosl,
                            op0=mybir.AluOpType.subtract,
                            op1=mybir.AluOpType.add)
                    nc.vector.max(cvb[:, d], pk[:])
                nc.sync.dma_start(cval_r[:, r], cvb[:])
    nc.compile()
    return nc


def _get_kernel():
    if "k" not in _cache:
        _cache["k"] = _build_sparse()
    return _cache["k"]


def prepare_inputs(x, W_enc, b_enc):
    """Build the per-core device input dicts from full f32 inputs."""
    import ml_dtypes

    fp8 = ml_dtypes.float8_e4m3

    # x8[p, rb, c2, ko, rr]: x is [N, K]; contraction index
    # k = c2*256 + ko*128 + p
    xT = np.ascontiguousarray(x.T)                       # [768, 4096]
    x8h = np.ascontiguousarray(
        xT.reshape(KP, 2, P, R_BLOCKS, R_BLK)
        .transpose(2, 3, 0, 1, 4)
        .reshape(P, R_BLOCKS * KP * 2 * R_BLK)).astype(fp8)
    ins = []
    for c in range(N_CORES):
        sl = slice(c * DICT_SH, (c + 1) * DICT_SH)
        # [p][d][c2][ko][m]: contraction k = c2*256 + ko*128 + p
        w8h = np.ascontiguousarray(
            (W_SCALE * W_enc[:, sl]).reshape(KP, 2, P, D_TILES, P)
            .transpose(2, 3, 0, 1, 4)
            .reshape(P, D_TILES * KP * 2 * P)).astype(fp8)
        bsh = (MROUND + QSCALE *
               np.ascontiguousarray(b_enc[sl]).reshape(D_TILES, P).T
               ).astype(np.float32)
        auxh = np.empty((P, 2 + D_TILES + R_BLK), np.float32)
        auxh[:, 0] = MROUND
        auxh[:, 1] = -MROUND
        auxh[:, 2:2 + D_TILES] = bsh
        auxh[:, 2 + D_TILES:] = (np.arange(R_BLK, dtype=np.float32)
                                 / 512.0)[None, :]
        ins.append({"x8": x8h, "w8": w8h, "aux": auxh})
    return ins


def _host_fallback(x, W_enc, b_enc, k_tot):
    """Exact dense path; only for degenerate/unexpected regimes."""
    z = x.astype(np.float32) @ W_enc.astype(np.float32) + b_enc[None, :]
    z = np.maximum(z, 0.0)
    flat = z.reshape(-1)
    idx = np.argpartition(-flat, k_tot - 1)[:k_tot]
    order = np.lexsort((idx, -flat[idx]))
    idx = idx[order]
    out = np.zeros_like(flat)
    out[idx] = flat[idx]
    return out.reshape(z.shape)


def kernel(x, W_enc, b_enc, top_k):
    from concourse.bass_utils import run_bass_kernel_spmd

    x = np.ascontiguousarray(np.asarray(x), np.float32)
    W_enc = np.ascontiguousarray(np.asarray(W_enc), np.float32)
    b_enc = np.ascontiguousarray(np.asarray(b_enc), np.float32).ravel()
    top_k = int(np.asarray(top_k))
    k_tot = top_k * x.shape[0]
    out = np.zeros((N_TOTAL, DICT), np.float32)
    if k_tot <= 0:
        return out

    nc = _get_kernel()
    ins = prepare_inputs(x, W_enc, b_enc)
    try:
        res = run_bass_kernel_spmd(nc, ins, core_ids=list(range(N_CORES)))
    except Exception:
        # transient device errors recover on re-execution; one retry
        res = run_bass_kernel_spmd(nc, ins, core_ids=list(range(N_CORES)))

    # ---- global merge (host) ----
    # flat layout: [core, rb, p, d, slot];  col = c*2048 + d*128 + p,
    # row = rb*512 + i;  packed = (q_z + b_q) + i/512
    vals = np.stack([res.results[c]["cval"] for c in range(N_CORES)])
    packed = vals.ravel().astype(np.float64)
    q = np.floor(packed)
    ii = np.rint((packed - q) * 512.0).astype(np.int64)
    vb = q / QSCALE

    n_flat = packed.size
    f = np.arange(n_flat, dtype=np.int64)
    c_, rem = np.divmod(f, R_BLOCKS * P * D_TILES * CW)
    rb, rem = np.divmod(rem, P * D_TILES * CW)
    p, rem = np.divmod(rem, D_TILES * CW)
    d, slot = np.divmod(rem, CW)
    col = (c_ * DICT_SH + d * P + p).astype(np.int64)
    row = rb * R_BLK + ii

    k_eff = min(k_tot, n_flat)
    tau_hat = float(np.partition(vb, -k_eff)[-k_eff])
    if tau_hat <= 2.0 * DELTA:
        # degenerate regime (huge k / tiny tau): device extraction cannot
        # cover the selection; fall back to the exact dense path.
        return _host_fallback(x, W_enc, b_enc, k_tot)

    thr = tau_hat - DELTA
    x64 = x.astype(np.float64)
    b64 = b_enc.astype(np.float64)

    # pool of exact values keyed by flat position
    pool_pos = []
    pool_val = []

    # 1. saturated chunks: 8th extracted value still above threshold ->
    #    recompute the whole (col, rb) chunk with per-rb BLAS GEMMs.
    v8 = vb[slot == 7]                       # per chunk (c, rb, p, d)
    sat_chunk = np.flatnonzero(v8 >= thr)
    if sat_chunk.size:
        sc_, srem = np.divmod(sat_chunk, R_BLOCKS * P * D_TILES)
        srb, srem = np.divmod(srem, P * D_TILES)
        sp, sd = np.divmod(srem, D_TILES)
        scol = sc_ * DICT_SH + sd * P + sp
        for rbi in range(R_BLOCKS):
            m = srb == rbi
            if not m.any():
                continue
            cols = np.unique(scol[m])
            xa = x64[rbi * R_BLK:(rbi + 1) * R_BLK]         # [512, 768]
            zc = xa @ W_enc[:, cols].astype(np.float64)     # [512, ncols]
            zc += b64[cols][None, :]
            rr, cc = np.nonzero(zc >= thr)
            if rr.size:
                pool_pos.append((rbi * R_BLK + rr) * DICT + cols[cc])
                pool_val.append(zc[rr, cc])
        chunk_id = f // CW
        in_sat_mask = np.isin(chunk_id, sat_chunk)
    else:
        in_sat_mask = np.zeros(n_flat, bool)

    # 2. remaining candidates above threshold: exact einsum recompute
    cand = (vb >= thr) & ~in_sat_mask
    er, ec = row[cand], col[cand]
    epos = er * DICT + ec
    epos, uq = np.unique(epos, return_index=True)
    er, ec = er[uq], ec[uq]
    ev = np.empty(er.size, np.float64)
    CH = 65536
    for i in range(0, er.size, CH):
        s = slice(i, i + CH)
        ev[s] = np.einsum(
            "ij,ij->i",
            x64[er[s]],
            W_enc[:, ec[s]].T.astype(np.float64)) + b64[ec[s]]
    pool_pos.append(epos)
    pool_val.append(ev)

    ppos = np.concatenate(pool_pos)
    pval = np.concatenate(pool_val)
    ppos, uq = np.unique(ppos, return_index=True)
    pval = pval[uq]

    if pval.size < k_tot:
        return _host_fallback(x, W_enc, b_enc, k_tot)

    # exact selection: value desc, flat index asc (matches jax.lax.top_k)
    order = np.lexsort((ppos, -pval))
    kept = order[:k_tot]
    kr, kc = np.divmod(ppos[kept], DICT)
    out[kr, kc] = np.maximum(pval[kept], 0.0).astype(np.float32)
    return out


# revision 28
# speedup vs baseline: 1.1355x; 1.1355x over previous
"""CrossLayerTranscoder with global batch-wise top-k masking on 8 TRN2 cores.

Reference computation:
    pre = relu(x @ W_enc + b_enc)            [4096, 16384]
    keep the global top-(top_k * 4096) entries, zero the rest.

Device algorithm (dict-sharded over 8 cores), per [128 cols x 512 rows] tile:
  * PE: GEMM in fp8(e4m3) with perf_mode=DoubleRow - each matmul absorbs
    K=256 contraction rows at ~2 rows/cycle, halving PE time vs bf16.
    W is pre-scaled by 32 (power of two) to center its values in the fp8
    normal range; the ACT scale divides it back out.  Differential noise
    on z is ~0.05 rms - all accuracy-critical values are recomputed
    exactly on the host.
  * ACT: a = 8*psum + (M + 256*b) stored f32; with M = 1.5*2^23 the store
    rounds to M + q exactly (ulp(M)=1), q = round(256*(z+b)).
  * pk = (a - M) + i/512: value-major, index-minor packing, exact in f32.
    Engine-split by d-parity to balance load: even d-tiles run it as one
    DVE scalar_tensor_tensor; odd d-tiles run ACT (a - M) then GPSIMD
    tensor_tensor (+i/512), freeing DVE for the MAX8 scans.  The three
    postprocess engines (ACT/DVE/GPSIMD) measure ~115/116/104us busy -
    the steady state is their balance point and paces the MM stream.
  * DVE: MAX8 per [128 cols x 512 rows] tile extracts the top-8 packed
    values per (dict col, 512-row block).
  * w8 is d-tile-major on the host so every weight DMA slice and every
    DoubleRow LDWEIGHTS read is contiguous per partition (a strided head
    transfer otherwise delays the first matmul by ~5us).
  * Host merge:
      - decode q+b_q = floor(packed), i = frac*512; v_hat = floor/256.
      - tau_hat = k-th largest candidate.
      - candidate pool: all candidates with v_hat >= tau_hat - DELTA, plus
        every entry of 'saturated' chunks (8th extracted value still >=
        tau_hat - DELTA).  Saturated chunks are recomputed with
        per-row-block BLAS GEMMs; the rest of the pool with a chunked
        einsum.  All pool values are EXACT (f64): both the selected set
        and the stored values come from exact arithmetic, so fp8 device
        noise never reaches the output.
"""

import numpy as np

P = 128
N_TOTAL = 4096
K_DIM = 768
DICT = 16384
N_CORES = 8
DICT_SH = DICT // N_CORES     # 2048
R_BLK = 512
R_BLOCKS = N_TOTAL // R_BLK   # 8
D_TILES = DICT_SH // P        # 16
D_PAIRS = D_TILES // 2        # 8
CW = 8                        # top-8 per (col, 512-row block)
KP = K_DIM // 256             # 3 DoubleRow k-pairs
DELTA = 0.26                  # band half-width (~5 sigma of fp8 noise)
MROUND = 1.5 * 2.0**23        # fp32 round-to-int magic constant
QSCALE = 256.0                # value quantization: q = round(256*z)
W_SCALE = 32.0                # fp8 pre-scale on W (power of 2)
GPS_MOD = 2                   # odd d-tiles pack on ACT+GPSIMD, even on DVE

_cache = {}


def _build_sparse():
    import concourse.mybir as mybir
    import concourse.tile as tile
    from concourse import bacc

    f32 = mybir.dt.float32
    fp8 = mybir.dt.float8e4

    nc = bacc.Bacc("TRN2", target_bir_lowering=False, debug=False,
                   num_devices=N_CORES)
    # host layouts (partition-major):
    #   x8[p, rb, c2, ko, rr]  = fp8(x[c2*256 + ko*128 + p, rb*512 + rr])
    #   w8[p, d, c2, ko, m]    = fp8(32 * W[c2*256 + ko*128 + p, d*128 + m])
    x8 = nc.dram_tensor("x8", [P, R_BLOCKS * KP * 2 * R_BLK], fp8,
                        kind="ExternalInput")
    w8 = nc.dram_tensor("w8", [P, KP * 2 * DICT_SH], fp8,
                        kind="ExternalInput")
    # aux[:, 0:2] = [+M, -M]; [:, 2:18] = per-d ACT bias; [:, 18:530] = iota
    aux = nc.dram_tensor("aux", [P, 2 + D_TILES + R_BLK], f32,
                         kind="ExternalInput")
    cval = nc.dram_tensor("cval", [R_BLOCKS * P, D_TILES * CW], f32,
                          kind="ExternalOutput")

    with tile.TileContext(nc) as tc:
        with (
            tc.tile_pool(name="resident", bufs=1) as rpool,
            tc.tile_pool(name="xstream", bufs=4) as xpool,
            tc.tile_pool(name="act", bufs=10) as apool,
            tc.tile_pool(name="pack", bufs=10) as ppool,
            tc.tile_pool(name="cand", bufs=2) as cpool,
            tc.tile_pool(name="psum", bufs=8, space="PSUM") as psum_pool,
        ):
            # w8 is d-tile-major: [p][d][c2][ko][m] - every DMA slice and
            # every LDWEIGHTS read is contiguous per partition
            w8_sb = rpool.tile([P, D_TILES, KP, 2, P], fp8)
            aux_sb = rpool.tile([P, 2 + D_TILES + R_BLK], f32)
            mm_sb = aux_sb[:, 0:2]
            b_sb = aux_sb[:, 2:2 + D_TILES]
            io_sb = aux_sb[:, 2 + D_TILES:]

            x8_r = x8.ap().rearrange("p (rb c k rr) -> p rb c k rr",
                                     c=KP, k=2, rr=R_BLK)
            w8_r = w8.ap().rearrange("p (d c k m) -> p d c k m",
                                     d=D_TILES, c=KP, k=2)
            cval_r = cval.ap().rearrange("(rb p) w -> p rb w", p=P)

            # priority-ordered head: first MMs need w8[d=0,1] and x8(r0)
            x0 = xpool.tile([P, KP, 2, R_BLK], fp8, tag="xh")
            nc.sync.dma_start(x0[:], x8_r[:, 0])
            nc.sync.dma_start(w8_sb[:, 0:2], w8_r[:, 0:2])
            nc.sync.dma_start(aux_sb[:], aux.ap())
            nc.sync.dma_start(w8_sb[:, 2:4], w8_r[:, 2:4])
            nc.sync.dma_start(w8_sb[:, 4:8], w8_r[:, 4:8])
            nc.sync.dma_start(w8_sb[:, 8:], w8_r[:, 8:])

            x_next = {0: x0}
            for r in range(R_BLOCKS):
                xh_t = x_next.pop(r)
                if r + 1 < R_BLOCKS:
                    xn = xpool.tile([P, KP, 2, R_BLK], fp8, tag="xh")
                    nc.sync.dma_start(xn[:], x8_r[:, r + 1])
                    x_next[r + 1] = xn
                cvb = cpool.tile([P, D_TILES, CW], f32, tag="cv")
                for d in range(D_TILES):
                    ps = psum_pool.tile([P, R_BLK], mybir.dt.float32)
                    for c2 in range(KP):
                        nc.tensor.matmul(
                            ps[:], w8_sb[:, d, c2], xh_t[:, c2],
                            start=(c2 == 0), stop=(c2 == KP - 1),
                            perf_mode=mybir.MatmulPerfMode.DoubleRow)
                    a_sb = apool.tile([P, R_BLK], f32, tag="a")
                    nc.scalar.activation(
                        a_sb[:], ps[:],
                        mybir.ActivationFunctionType.Identity,
                        bias=b_sb[:, d:d + 1], scale=QSCALE / W_SCALE)
                    pk = ppool.tile([P, R_BLK], f32, tag="pk")
                    iosl = io_sb[:]
                    if d % GPS_MOD == 1:
                        # ACT removes M, GPSIMD adds bias+iota
                        a2 = apool.tile([P, R_BLK], f32, tag="a2")
                        nc.scalar.activation(
                            a2[:], a_sb[:],
                            mybir.ActivationFunctionType.Identity,
                            bias=mm_sb[:, 1:2], scale=1.0)
                        nc.gpsimd.tensor_tensor(
                            pk[:], a2[:], iosl, op=mybir.AluOpType.add)
                    else:
                        nc.vector.scalar_tensor_tensor(
                            pk[:], a_sb[:], MROUND, iosl,
                            op0=mybir.AluOpType.subtract,
                            op1=mybir.AluOpType.add)
                    nc.vector.max(cvb[:, d], pk[:])
                nc.sync.dma_start(cval_r[:, r], cvb[:])
    nc.compile()
    return nc


def _get_kernel():
    if "k" not in _cache:
        _cache["k"] = _build_sparse()
    return _cache["k"]


def prepare_inputs(x, W_enc, b_enc):
    """Build the per-core device input dicts from full f32 inputs."""
    import ml_dtypes

    fp8 = ml_dtypes.float8_e4m3

    # x8[p, rb, c2, ko, rr]: x is [N, K]; contraction index
    # k = c2*256 + ko*128 + p
    xT = np.ascontiguousarray(x.T)                       # [768, 4096]
    x8h = np.ascontiguousarray(
        xT.reshape(KP, 2, P, R_BLOCKS, R_BLK)
        .transpose(2, 3, 0, 1, 4)
        .reshape(P, R_BLOCKS * KP * 2 * R_BLK)).astype(fp8)
    ins = []
    for c in range(N_CORES):
        sl = slice(c * DICT_SH, (c + 1) * DICT_SH)
        # [p][d][c2][ko][m]: contraction k = c2*256 + ko*128 + p
        w8h = np.ascontiguousarray(
            (W_SCALE * W_enc[:, sl]).reshape(KP, 2, P, D_TILES, P)
            .transpose(2, 3, 0, 1, 4)
            .reshape(P, D_TILES * KP * 2 * P)).astype(fp8)
        bsh = (MROUND + QSCALE *
               np.ascontiguousarray(b_enc[sl]).reshape(D_TILES, P).T
               ).astype(np.float32)
        auxh = np.empty((P, 2 + D_TILES + R_BLK), np.float32)
        auxh[:, 0] = MROUND
        auxh[:, 1] = -MROUND
        auxh[:, 2:2 + D_TILES] = bsh
        auxh[:, 2 + D_TILES:] = (np.arange(R_BLK, dtype=np.float32)
                                 / 512.0)[None, :]
        ins.append({"x8": x8h, "w8": w8h, "aux": auxh})
    return ins


def _host_fallback(x, W_enc, b_enc, k_tot):
    """Exact dense path; only for degenerate/unexpected regimes."""
    z = x.astype(np.float32) @ W_enc.astype(np.float32) + b_enc[None, :]
    z = np.maximum(z, 0.0)
    flat = z.reshape(-1)
    idx = np.argpartition(-flat, k_tot - 1)[:k_tot]
    order = np.lexsort((idx, -flat[idx]))
    idx = idx[order]
    out = np.zeros_like(flat)
    out[idx] = flat[idx]
    return out.reshape(z.shape)


def kernel(x, W_enc, b_enc, top_k):
    from concourse.bass_utils import run_bass_kernel_spmd

    x = np.ascontiguousarray(np.asarray(x), np.float32)
    W_enc = np.ascontiguousarray(np.asarray(W_enc), np.float32)
    b_enc = np.ascontiguousarray(np.asarray(b_enc), np.float32).ravel()
    top_k = int(np.asarray(top_k))
    k_tot = top_k * x.shape[0]
    out = np.zeros((N_TOTAL, DICT), np.float32)
    if k_tot <= 0:
        return out

    nc = _get_kernel()
    ins = prepare_inputs(x, W_enc, b_enc)
    try:
        res = run_bass_kernel_spmd(nc, ins, core_ids=list(range(N_CORES)))
    except Exception:
        # transient device errors recover on re-execution; one retry
        res = run_bass_kernel_spmd(nc, ins, core_ids=list(range(N_CORES)))

    # ---- global merge (host) ----
    # flat layout: [core, rb, p, d, slot];  col = c*2048 + d*128 + p,
    # row = rb*512 + i;  packed = (q_z + b_q) + i/512
    vals = np.stack([res.results[c]["cval"] for c in range(N_CORES)])
    packed = vals.ravel().astype(np.float64)
    q = np.floor(packed)
    ii = np.rint((packed - q) * 512.0).astype(np.int64)
    vb = q / QSCALE

    n_flat = packed.size
    f = np.arange(n_flat, dtype=np.int64)
    c_, rem = np.divmod(f, R_BLOCKS * P * D_TILES * CW)
    rb, rem = np.divmod(rem, P * D_TILES * CW)
    p, rem = np.divmod(rem, D_TILES * CW)
    d, slot = np.divmod(rem, CW)
    col = (c_ * DICT_SH + d * P + p).astype(np.int64)
    row = rb * R_BLK + ii

    k_eff = min(k_tot, n_flat)
    tau_hat = float(np.partition(vb, -k_eff)[-k_eff])
    if tau_hat <= 2.0 * DELTA:
        # degenerate regime (huge k / tiny tau): device extraction cannot
        # cover the selection; fall back to the exact dense path.
        return _host_fallback(x, W_enc, b_enc, k_tot)

    thr = tau_hat - DELTA
    x64 = x.astype(np.float64)
    b64 = b_enc.astype(np.float64)

    # pool of exact values keyed by flat position
    pool_pos = []
    pool_val = []

    # 1. saturated chunks: 8th extracted value still above threshold ->
    #    recompute the whole (col, rb) chunk with per-rb BLAS GEMMs.
    v8 = vb[slot == 7]                       # per chunk (c, rb, p, d)
    sat_chunk = np.flatnonzero(v8 >= thr)
    if sat_chunk.size:
        sc_, srem = np.divmod(sat_chunk, R_BLOCKS * P * D_TILES)
        srb, srem = np.divmod(srem, P * D_TILES)
        sp, sd = np.divmod(srem, D_TILES)
        scol = sc_ * DICT_SH + sd * P + sp
        for rbi in range(R_BLOCKS):
            m = srb == rbi
            if not m.any():
                continue
            cols = np.unique(scol[m])
            xa = x64[rbi * R_BLK:(rbi + 1) * R_BLK]         # [512, 768]
            zc = xa @ W_enc[:, cols].astype(np.float64)     # [512, ncols]
            zc += b64[cols][None, :]
            rr, cc = np.nonzero(zc >= thr)
            if rr.size:
                pool_pos.append((rbi * R_BLK + rr) * DICT + cols[cc])
                pool_val.append(zc[rr, cc])
        chunk_id = f // CW
        in_sat_mask = np.isin(chunk_id, sat_chunk)
    else:
        in_sat_mask = np.zeros(n_flat, bool)

    # 2. remaining candidates above threshold: exact einsum recompute
    cand = (vb >= thr) & ~in_sat_mask
    er, ec = row[cand], col[cand]
    epos = er * DICT + ec
    epos, uq = np.unique(epos, return_index=True)
    er, ec = er[uq], ec[uq]
    ev = np.empty(er.size, np.float64)
    CH = 65536
    for i in range(0, er.size, CH):
        s = slice(i, i + CH)
        ev[s] = np.einsum(
            "ij,ij->i",
            x64[er[s]],
            W_enc[:, ec[s]].T.astype(np.float64)) + b64[ec[s]]
    pool_pos.append(epos)
    pool_val.append(ev)

    ppos = np.concatenate(pool_pos)
    pval = np.concatenate(pool_val)
    ppos, uq = np.unique(ppos, return_index=True)
    pval = pval[uq]

    if pval.size < k_tot:
        return _host_fallback(x, W_enc, b_enc, k_tot)

    # exact selection: value desc, flat index asc (matches jax.lax.top_k)
    order = np.lexsort((ppos, -pval))
    kept = order[:k_tot]
    kr, kc = np.divmod(ppos[kept], DICT)
    out[kr, kc] = np.maximum(pval[kept], 0.0).astype(np.float32)
    return out
